# revision 6
# baseline (speedup 1.0000x reference)
"""Trainium2 Bass kernel for CombinedPriorityLoss (MSE + pairwise ranking + diversity).

v7: mask-times-value products and their reduction both happen on TensorE via
diagonal product-matmuls; DVE only makes 0/1 masks and relu values (all
tensor_scalar at 4x bf16 -- no tensor_tensor at all); ACT only computes dt^2.

Math (all-ordered-pairs symmetric form; m = MARGIN):
  rank * paircount = sum_{i,j} [ c1_ij * relu(m - dp) + cmid_ij * (d+ + d-) ]
  with c1 = 1[t_j < t_i - m], cmid = 1[(t_i - t_j)^2 <= m^2], dp = p_i - p_j,
  d+ = relu(0.05*dp), d- = relu(-0.05*dp)  (so d+ + d- = 0.05*|dp|).

For each 128-column chunk c, TensorE computes
  psum[m, n] += sum_i mask[i, c+m] * value[i, c+n]
for the three (mask, value) pairs (c1, ra), (cmid, d+), (cmid, d-), all
accumulated into ONE [128, 128] PSUM tile. Only the diagonal (m == n) is
meaningful; a final scalar_tensor_tensor multiply with an identity matrix
extracts and sums it (off-diagonal garbage is simply never read).

Layout: rows i on partitions (RB=8 blocks of 128 per core), cols j on the free
dim (2 chunks of F=4096).
"""

import numpy as np
import ml_dtypes

import concourse.bacc as bacc
import concourse.mybir as mybir
from concourse.tile import TileContext
from concourse.bass_utils import run_bass_kernel_spmd

N = 8192
N_CORES = 8
ROWS_PER_CORE = N // N_CORES          # 1024
RB = ROWS_PER_CORE // 128             # 8 row blocks per core
F = 4096                              # j-chunk width
NC_J = N // F                         # 2 chunks
MARGIN = 0.2
MSE_W = 0.1
RANK_W = 0.9
DIV_W = 0.1
BIG = float(2.0 ** 20)
FP = 3328                             # dN columns computed on Pool (gpsimd)

F32 = mybir.dt.float32
BF16 = mybir.dt.bfloat16
Alu = mybir.AluOpType
Act = mybir.ActivationFunctionType


def _build(reps: int = 1):
    nacc = 2

    nc = bacc.Bacc(None)
    p05_bf = nc.dram_tensor("p05_bf", [N], BF16, kind="ExternalInput")
    p05n_bf = nc.dram_tensor("p05n_bf", [N], BF16, kind="ExternalInput")
    tcol_bf = nc.dram_tensor("tcol_bf", [N], BF16, kind="ExternalInput")
    ident = nc.dram_tensor("ident", [128, 128], BF16, kind="ExternalInput")
    prow = nc.dram_tensor("prow", [ROWS_PER_CORE], F32, kind="ExternalInput")
    trow = nc.dram_tensor("trow", [ROWS_PER_CORE], F32, kind="ExternalInput")
    accC_d = nc.dram_tensor("accC", [128, nacc], F32, kind="ExternalOutput")
    stats_d = nc.dram_tensor("stats", [128, 5], F32, kind="ExternalOutput")

    with TileContext(nc) as tc:
        with (
            tc.tile_pool(name="bcast", bufs=1) as bpool,
            tc.tile_pool(name="rows", bufs=1) as rpool,
            tc.tile_pool(name="work", bufs=3) as wpool,
            tc.tile_pool(name="accs", bufs=1) as apool,
            tc.tile_pool(name="psum", bufs=1, space="PSUM") as ppool,
        ):
            # --- row scalars first (tiny DMAs; unblock scalar setup + stats) ---
            prow_t = rpool.tile([128, RB], F32, name="prow_t")
            trow_t = rpool.tile([128, RB], F32, name="trow_t")
            nc.sync.dma_start(prow_t[:, :], prow.rearrange("(rb p) -> p rb", p=128))
            nc.sync.dma_start(trow_t[:, :], trow.rearrange("(rb p) -> p rb", p=128))

            # --- broadcast column tiles, grouped so the jc=0 halves land first ---
            p05_b = bpool.tile([128, N], BF16, name="p05_b")
            p05n_b = bpool.tile([128, N], BF16, name="p05n_b")
            tcol_b = bpool.tile([128, N], BF16, name="tcol_b")
            nchunk = 8
            csz = N // nchunk
            for jc in range(NC_J):
                rng = range(jc * (nchunk // NC_J), (jc + 1) * (nchunk // NC_J))
                for src, dst in ((tcol_bf, tcol_b),
                                 (p05_bf, p05_b), (p05n_bf, p05n_b)):
                    for i in rng:
                        sl = slice(i * csz, (i + 1) * csz)
                        nc.sync.dma_start(dst[:, sl],
                                          src[None, sl].partition_broadcast(128))

            # identity matrix for the PSUM diagonal extraction
            ident_t = bpool.tile([128, 128], BF16, name="ident_t")
            nc.sync.dma_start(ident_t[:, :], ident[:, :])

            # per-rb per-partition scalars
            s1 = rpool.tile([128, RB], F32, name="s1")        # t_i - m    (gate1 threshold)
            s05r = rpool.tile([128, RB], F32, name="s05r")    # 0.05*(m - p_i)
            s05 = rpool.tile([128, RB], F32, name="s05")      # 0.05*p_i
            tneg = rpool.tile([128, RB], F32, name="tneg")    # -t_i       (ACT square bias)
            nc.vector.tensor_scalar(s1[:, :], trow_t[:, :], -MARGIN, None, Alu.add)
            nc.vector.tensor_scalar(s05r[:, :], prow_t[:, :], -0.05, 0.05 * MARGIN,
                                    Alu.mult, Alu.add)
            nc.vector.tensor_scalar(s05[:, :], prow_t[:, :], 0.05, None, Alu.mult)
            nc.vector.tensor_scalar(tneg[:, :], trow_t[:, :], -1.0, None, Alu.mult)

            # --- O(N) stats on this core's row slice ---
            stats_t = apool.tile([128, 5], F32, name="stats_t")
            d_t = rpool.tile([128, RB], F32, name="d_t")
            nc.vector.scalar_tensor_tensor(d_t[:, :], prow_t[:, :], 1.0, trow_t[:, :],
                                           Alu.mult, Alu.subtract)
            scr = rpool.tile([128, RB], F32, name="scr")
            nc.vector.scalar_tensor_tensor(scr[:, :], d_t[:, :], 1.0, d_t[:, :],
                                           Alu.mult, Alu.mult, accum_out=stats_t[:, 0:1])
            nc.vector.scalar_tensor_tensor(scr[:, :], prow_t[:, :], 1.0, prow_t[:, :],
                                           Alu.mult, Alu.mult, accum_out=stats_t[:, 1:2])
            nc.vector.scalar_tensor_tensor(scr[:, :], trow_t[:, :], 1.0, trow_t[:, :],
                                           Alu.mult, Alu.mult, accum_out=stats_t[:, 2:3])
            nc.vector.tensor_scalar(scr[:, :], prow_t[:, :], 1.0, 0.0, Alu.mult, Alu.add,
                                    accum_out=stats_t[:, 3:4])
            nc.vector.tensor_scalar(scr[:, :], trow_t[:, :], 1.0, 0.0, Alu.mult, Alu.add,
                                    accum_out=stats_t[:, 4:5])

            # --- main pairwise loop (hardware loop over reps) ---
            accC = apool.tile([128, nacc], F32, name="accC_t")
            scr128 = apool.tile([128, 128], BF16, name="scr128")
            psumA = ppool.tile([128, 128], F32, name="psumA")
            psumB = ppool.tile([128, 128], F32, name="psumB")
            n_ch = F // 128
            last = (RB - 1, NC_J - 1, n_ch - 1)
            with tc.For_i(0, reps) as _rep:
                for rb in range(RB):
                    for jc in range(NC_J):
                        cs = slice(jc * F, (jc + 1) * F)
                        m1 = wpool.tile([128, F], BF16, name="m1")
                        ra = wpool.tile([128, F], BF16, name="ra")
                        dP = wpool.tile([128, F], BF16, name="dP")
                        dN = wpool.tile([128, F], BF16, name="dN")
                        SQ = wpool.tile([128, F], BF16, name="SQ")
                        m2 = SQ
                        # dt^2 on ACT
                        nc.scalar.activation(SQ[:, :], tcol_b[:, cs], Act.Square,
                                             bias=tneg[:, rb:rb + 1], scale=1.0)
                        # exact 0/1 masks on DVE (4x bf16)
                        nc.vector.tensor_scalar(m1[:, :], tcol_b[:, cs],
                                                s1[:, rb:rb + 1], None, Alu.is_lt)
                        nc.vector.tensor_scalar(m2[:, :], SQ[:, :],
                                                MARGIN * MARGIN, None, Alu.is_le)
                        # values on DVE (4x bf16): relu(m-dp), relu(.05dp), relu(-.05dp)
                        nc.vector.tensor_scalar(ra[:, :], p05_b[:, cs],
                                                s05r[:, rb:rb + 1], 0.0,
                                                Alu.add, Alu.max)
                        nc.vector.tensor_scalar(dP[:, :], p05n_b[:, cs],
                                                s05[:, rb:rb + 1], 0.0,
                                                Alu.add, Alu.max)
                        nc.gpsimd.tensor_scalar(dN[:, 0:FP],
                                                p05_b[:, jc * F:jc * F + FP],
                                                s05[:, rb:rb + 1], 0.0,
                                                Alu.subtract, Alu.max)
                        nc.vector.tensor_scalar(dN[:, FP:F],
                                                p05_b[:, jc * F + FP:(jc + 1) * F],
                                                s05[:, rb:rb + 1], 0.0,
                                                Alu.subtract, Alu.max)
                        # TensorE: psum[m, n] += sum_i mask[i, m]*value[i, n]
                        # for the three (mask, value) pairs; diagonal is the
                        # masked pairwise sum. One accumulation group per rep.
                        for c in range(n_ch):
                            cc = slice(c * 128, (c + 1) * 128)
                            first = (rb, jc, c) == (0, 0, 0)
                            is_last = (rb, jc, c) == last
                            nc.tensor.matmul(psumA[:, :], m1[:, cc], ra[:, cc],
                                             start=first, stop=is_last)
                            nc.tensor.matmul(psumB[:, :], m2[:, cc], dP[:, cc],
                                             start=first, stop=False)
                            nc.tensor.matmul(psumB[:, :], m2[:, cc], dN[:, cc],
                                             start=False, stop=is_last)
            # extract + sum the PSUM diagonal (once -- every rep leaves the
            # same totals in PSUM, so this lives outside the hardware loop and
            # never stalls the next rep's accumulation group)
            nc.vector.scalar_tensor_tensor(scr128[:, :], psumA[:, :], 1.0,
                                           ident_t[:, :], Alu.mult, Alu.mult,
                                           accum_out=accC[:, 0:1])
            nc.vector.scalar_tensor_tensor(scr128[:, :], psumB[:, :], 1.0,
                                           ident_t[:, :], Alu.mult, Alu.mult,
                                           accum_out=accC[:, 1:2])

            nc.sync.dma_start(accC_d[:, :], accC[:, :])
            nc.sync.dma_start(stats_d[:, :], stats_t[:, :])

    nc.compile()
    return nc


_NC_CACHE = {}


def _get_nc(reps: int = 1):
    if reps not in _NC_CACHE:
        _NC_CACHE[reps] = _build(reps=reps)
    return _NC_CACHE[reps]


class _CachedRunner:
    """Build the shard_map-jitted bass_exec callable once, reuse across calls."""

    def __init__(self, nc):
        import jax
        from jax.experimental.shard_map import shard_map
        from jax.sharding import Mesh, PartitionSpec
        from concourse import bass2jax, mybir as _mybir

        bass2jax.install_neuronx_cc_hook()
        self.nc = nc
        in_names, out_names, out_avals = [], [], []
        partition_name = (nc.partition_id_tensor.name
                          if nc.partition_id_tensor else None)
        for alloc in nc.m.functions[0].allocations:
            if not isinstance(alloc, _mybir.MemoryLocationSet):
                continue
            name = alloc.memorylocations[0].name
            if alloc.kind == "ExternalInput":
                if name != partition_name:
                    in_names.append(name)
            elif alloc.kind == "ExternalOutput":
                out_avals.append(jax.core.ShapedArray(
                    tuple(alloc.tensor_shape), _mybir.dt.np(alloc.dtype)))
                out_names.append(name)
        self.in_names, self.out_names, self.out_avals = in_names, out_names, out_avals
        n_params, n_outs = len(in_names), len(out_names)
        self.n_params = n_params
        all_names = in_names + out_names + ([partition_name] if partition_name else [])

        def _body(*args):
            operands = list(args)
            if partition_name is not None:
                operands.append(bass2jax.partition_id_tensor())
            return tuple(bass2jax._bass_exec_p.bind(
                *operands,
                out_avals=tuple(out_avals),
                in_names=tuple(all_names),
                out_names=tuple(out_names),
                lowering_input_output_aliases=(),
                sim_require_finite=True,
                sim_require_nnan=True,
                nc=nc,
            ))

        devices = jax.devices()[:N_CORES]
        mesh = Mesh(np.asarray(devices), ("core",))
        in_specs = (PartitionSpec("core"),) * (n_params + n_outs)
        out_specs = (PartitionSpec("core"),) * n_outs
        self.fn = jax.jit(
            shard_map(_body, mesh=mesh, in_specs=in_specs, out_specs=out_specs,
                      check_rep=False),
            donate_argnums=tuple(range(n_params, n_params + n_outs)),
            keep_unused=True,
        )

    def __call__(self, in_maps):
        concat_in = [
            np.concatenate([np.asarray(m[name]) for m in in_maps], axis=0)
            for name in self.in_names
        ]
        concat_zeros = [
            np.zeros((N_CORES * a.shape[0], *a.shape[1:]), a.dtype)
            for a in self.out_avals
        ]
        out_arrs = self.fn(*concat_in, *concat_zeros)
        import jax
        jax.block_until_ready(out_arrs)
        return [
            {name: np.asarray(out_arrs[i]).reshape(
                N_CORES, *self.out_avals[i].shape)[c]
             for i, name in enumerate(self.out_names)}
            for c in range(N_CORES)
        ]


_RUNNER_CACHE = {}


def _get_runner(reps: int = 1):
    if reps not in _RUNNER_CACHE:
        _RUNNER_CACHE[reps] = _CachedRunner(_get_nc(reps))
    return _RUNNER_CACHE[reps]


def _in_maps(p: np.ndarray, t: np.ndarray):
    t_bf = t.astype(ml_dtypes.bfloat16)
    p_bf = p.astype(ml_dtypes.bfloat16)
    p05_bf = (0.05 * p).astype(ml_dtypes.bfloat16)
    p05n_bf = (-0.05 * p).astype(ml_dtypes.bfloat16)
    ident = np.eye(128, dtype=ml_dtypes.bfloat16)
    in_maps = []
    for c in range(N_CORES):
        rs = slice(c * ROWS_PER_CORE, (c + 1) * ROWS_PER_CORE)
        in_maps.append({
            "tcol_bf": t_bf,
            "p05_bf": p05_bf, "p05n_bf": p05n_bf, "ident": ident,
            "prow": np.ascontiguousarray(p[rs]),
            "trow": np.ascontiguousarray(t[rs]),
        })
    return in_maps


def _run(nc, p: np.ndarray, t: np.ndarray):
    return run_bass_kernel_spmd(nc, _in_maps(p, t), core_ids=list(range(N_CORES)))


def _combine(results) -> np.float32:
    SA = 0.0
    SB = 0.0
    s_d2 = s_p2 = s_t2 = s_p = s_t = 0.0
    for r in results:
        acc = r["accC"].astype(np.float64)
        SA += acc[:, 0].sum()
        SB += acc[:, 1].sum()
        st = r["stats"].astype(np.float64)
        s_d2 += st[:, 0].sum()
        s_p2 += st[:, 1].sum()
        s_t2 += st[:, 2].sum()
        s_p += st[:, 3].sum()
        s_t += st[:, 4].sum()

    pair_count = N * (N - 1) // 2
    rank = (20.0 * SA + SB) / pair_count
    mse = s_d2 / N
    var_p = (s_p2 - s_p * s_p / N) / (N - 1)
    var_t = (s_t2 - s_t * s_t / N) / (N - 1)
    div = max(var_t - var_p, 0.0)
    return np.float32(MSE_W * mse + RANK_W * rank + DIV_W * div)


def kernel(predictions, targets) -> np.ndarray:
    p = np.asarray(predictions, dtype=np.float32)
    t = np.asarray(targets, dtype=np.float32)
    runner = _get_runner(reps=1)
    results = runner(_in_maps(p, t))
    out = _combine(results)
    return np.asarray(out, dtype=np.float32)


# revision 7
# speedup vs baseline: 9.1800x; 9.1800x over previous
"""Trainium2 Bass kernel for CombinedPriorityLoss (MSE + pairwise ranking + diversity).

v7: mask-times-value products and their reduction both happen on TensorE via
diagonal product-matmuls; DVE only makes 0/1 masks and relu values (all
tensor_scalar at 4x bf16 -- no tensor_tensor at all); ACT only computes dt^2.

Math (all-ordered-pairs symmetric form; m = MARGIN):
  rank * paircount = sum_{i,j} [ c1_ij * relu(m - dp) + cmid_ij * (d+ + d-) ]
  with c1 = 1[t_j < t_i - m], cmid = 1[(t_i - t_j)^2 <= m^2], dp = p_i - p_j,
  d+ = relu(0.05*dp), d- = relu(-0.05*dp)  (so d+ + d- = 0.05*|dp|).

For each 128-column chunk c, TensorE computes
  psum[m, n] += sum_i mask[i, c+m] * value[i, c+n]
for the three (mask, value) pairs (c1, ra), (cmid, d+), (cmid, d-), all
accumulated into ONE [128, 128] PSUM tile. Only the diagonal (m == n) is
meaningful; a final scalar_tensor_tensor multiply with an identity matrix
extracts and sums it (off-diagonal garbage is simply never read).

Layout: rows i on partitions (RB=8 blocks of 128 per core), cols j on the free
dim (2 chunks of F=4096).
"""

import numpy as np
import ml_dtypes

import concourse.bacc as bacc
import concourse.mybir as mybir
from concourse.tile import TileContext
from concourse.bass_utils import run_bass_kernel_spmd

N = 8192
N_CORES = 8
ROWS_PER_CORE = N // N_CORES          # 1024
RB = ROWS_PER_CORE // 128             # 8 row blocks per core
F = 4096                              # j-chunk width
NC_J = N // F                         # 2 chunks
MARGIN = 0.2
MSE_W = 0.1
RANK_W = 0.9
DIV_W = 0.1
BIG = float(2.0 ** 20)

F32 = mybir.dt.float32
BF16 = mybir.dt.bfloat16
Alu = mybir.AluOpType
Act = mybir.ActivationFunctionType


def _build(reps: int = 1):
    nacc = 2

    nc = bacc.Bacc(None)
    p05_bf = nc.dram_tensor("p05_bf", [N], BF16, kind="ExternalInput")
    p05n_bf = nc.dram_tensor("p05n_bf", [N], BF16, kind="ExternalInput")
    tcol_bf = nc.dram_tensor("tcol_bf", [N], BF16, kind="ExternalInput")
    ident = nc.dram_tensor("ident", [128, 128], BF16, kind="ExternalInput")
    prow = nc.dram_tensor("prow", [ROWS_PER_CORE], F32, kind="ExternalInput")
    trow = nc.dram_tensor("trow", [ROWS_PER_CORE], F32, kind="ExternalInput")
    accC_d = nc.dram_tensor("accC", [128, nacc], F32, kind="ExternalOutput")
    stats_d = nc.dram_tensor("stats", [128, 5], F32, kind="ExternalOutput")

    with TileContext(nc) as tc:
        with (
            tc.tile_pool(name="bcast", bufs=1) as bpool,
            tc.tile_pool(name="rows", bufs=1) as rpool,
            tc.tile_pool(name="work", bufs=3) as wpool,
            tc.tile_pool(name="accs", bufs=1) as apool,
            tc.tile_pool(name="psum", bufs=1, space="PSUM") as ppool,
        ):
            # --- row scalars first (tiny DMAs; unblock scalar setup + stats) ---
            prow_t = rpool.tile([128, RB], F32, name="prow_t")
            trow_t = rpool.tile([128, RB], F32, name="trow_t")
            nc.sync.dma_start(prow_t[:, :], prow.rearrange("(rb p) -> p rb", p=128))
            nc.sync.dma_start(trow_t[:, :], trow.rearrange("(rb p) -> p rb", p=128))

            # --- broadcast column tiles, grouped so the jc=0 halves land first ---
            p05_b = bpool.tile([128, N], BF16, name="p05_b")
            p05n_b = bpool.tile([128, N], BF16, name="p05n_b")
            tcol_b = bpool.tile([128, N], BF16, name="tcol_b")
            nchunk = 8
            csz = N // nchunk
            for jc in range(NC_J):
                rng = range(jc * (nchunk // NC_J), (jc + 1) * (nchunk // NC_J))
                for src, dst in ((tcol_bf, tcol_b),
                                 (p05_bf, p05_b), (p05n_bf, p05n_b)):
                    for i in rng:
                        sl = slice(i * csz, (i + 1) * csz)
                        nc.sync.dma_start(dst[:, sl],
                                          src[None, sl].partition_broadcast(128))

            # identity matrix for the PSUM diagonal extraction
            ident_t = bpool.tile([128, 128], BF16, name="ident_t")
            nc.sync.dma_start(ident_t[:, :], ident[:, :])

            # per-rb per-partition scalars
            s1 = rpool.tile([128, RB], F32, name="s1")        # t_i - m    (gate1 threshold)
            s05r = rpool.tile([128, RB], F32, name="s05r")    # 0.05*(m - p_i)
            s05 = rpool.tile([128, RB], F32, name="s05")      # 0.05*p_i
            tneg = rpool.tile([128, RB], F32, name="tneg")    # -t_i       (ACT square bias)
            nc.vector.tensor_scalar(s1[:, :], trow_t[:, :], -MARGIN, None, Alu.add)
            nc.vector.tensor_scalar(s05r[:, :], prow_t[:, :], -0.05, 0.05 * MARGIN,
                                    Alu.mult, Alu.add)
            nc.vector.tensor_scalar(s05[:, :], prow_t[:, :], 0.05, None, Alu.mult)
            nc.vector.tensor_scalar(tneg[:, :], trow_t[:, :], -1.0, None, Alu.mult)

            # --- O(N) stats on this core's row slice ---
            stats_t = apool.tile([128, 5], F32, name="stats_t")
            d_t = rpool.tile([128, RB], F32, name="d_t")
            nc.vector.scalar_tensor_tensor(d_t[:, :], prow_t[:, :], 1.0, trow_t[:, :],
                                           Alu.mult, Alu.subtract)
            scr = rpool.tile([128, RB], F32, name="scr")
            nc.vector.scalar_tensor_tensor(scr[:, :], d_t[:, :], 1.0, d_t[:, :],
                                           Alu.mult, Alu.mult, accum_out=stats_t[:, 0:1])
            nc.vector.scalar_tensor_tensor(scr[:, :], prow_t[:, :], 1.0, prow_t[:, :],
                                           Alu.mult, Alu.mult, accum_out=stats_t[:, 1:2])
            nc.vector.scalar_tensor_tensor(scr[:, :], trow_t[:, :], 1.0, trow_t[:, :],
                                           Alu.mult, Alu.mult, accum_out=stats_t[:, 2:3])
            nc.vector.tensor_scalar(scr[:, :], prow_t[:, :], 1.0, 0.0, Alu.mult, Alu.add,
                                    accum_out=stats_t[:, 3:4])
            nc.vector.tensor_scalar(scr[:, :], trow_t[:, :], 1.0, 0.0, Alu.mult, Alu.add,
                                    accum_out=stats_t[:, 4:5])

            # --- main pairwise loop (hardware loop over reps) ---
            accC = apool.tile([128, nacc], F32, name="accC_t")
            scr128 = apool.tile([128, 128], BF16, name="scr128")
            psumA = ppool.tile([128, 128], F32, name="psumA")
            psumB = ppool.tile([128, 128], F32, name="psumB")
            n_ch = F // 128
            last = (RB - 1, NC_J - 1, n_ch - 1)
            with tc.For_i(0, reps) as _rep:
                for rb in range(RB):
                    for jc in range(NC_J):
                        cs = slice(jc * F, (jc + 1) * F)
                        m1 = wpool.tile([128, F], BF16, name="m1")
                        ra = wpool.tile([128, F], BF16, name="ra")
                        dP = wpool.tile([128, F], BF16, name="dP")
                        dN = wpool.tile([128, F], BF16, name="dN")
                        SQ = wpool.tile([128, F], BF16, name="SQ")
                        m2 = SQ
                        # dt^2 on ACT
                        nc.scalar.activation(SQ[:, :], tcol_b[:, cs], Act.Square,
                                             bias=tneg[:, rb:rb + 1], scale=1.0)
                        # exact 0/1 masks on DVE (4x bf16)
                        nc.vector.tensor_scalar(m1[:, :], tcol_b[:, cs],
                                                s1[:, rb:rb + 1], None, Alu.is_lt)
                        nc.vector.tensor_scalar(m2[:, :], SQ[:, :],
                                                MARGIN * MARGIN, None, Alu.is_le)
                        # values on DVE (4x bf16): relu(m-dp), relu(.05dp), relu(-.05dp)
                        nc.vector.tensor_scalar(ra[:, :], p05_b[:, cs],
                                                s05r[:, rb:rb + 1], 0.0,
                                                Alu.add, Alu.max)
                        nc.vector.tensor_scalar(dP[:, :], p05n_b[:, cs],
                                                s05[:, rb:rb + 1], 0.0,
                                                Alu.add, Alu.max)
                        nc.vector.tensor_scalar(dN[:, :], p05_b[:, cs],
                                                s05[:, rb:rb + 1], 0.0,
                                                Alu.subtract, Alu.max)
                        # TensorE: psum[m, n] += sum_i mask[i, m]*value[i, n]
                        # for the three (mask, value) pairs; diagonal is the
                        # masked pairwise sum. One accumulation group per rep.
                        for c in range(n_ch):
                            cc = slice(c * 128, (c + 1) * 128)
                            first = (rb, jc, c) == (0, 0, 0)
                            is_last = (rb, jc, c) == last
                            nc.tensor.matmul(psumA[:, :], m1[:, cc], ra[:, cc],
                                             start=first, stop=is_last)
                            nc.tensor.matmul(psumB[:, :], m2[:, cc], dP[:, cc],
                                             start=first, stop=False)
                            nc.tensor.matmul(psumB[:, :], m2[:, cc], dN[:, cc],
                                             start=False, stop=is_last)
            # extract + sum the PSUM diagonal (once -- every rep leaves the
            # same totals in PSUM, so this lives outside the hardware loop and
            # never stalls the next rep's accumulation group)
            nc.vector.scalar_tensor_tensor(scr128[:, :], psumA[:, :], 1.0,
                                           ident_t[:, :], Alu.mult, Alu.mult,
                                           accum_out=accC[:, 0:1])
            nc.vector.scalar_tensor_tensor(scr128[:, :], psumB[:, :], 1.0,
                                           ident_t[:, :], Alu.mult, Alu.mult,
                                           accum_out=accC[:, 1:2])

            nc.sync.dma_start(accC_d[:, :], accC[:, :])
            nc.sync.dma_start(stats_d[:, :], stats_t[:, :])

    nc.compile()
    return nc


_NC_CACHE = {}


def _get_nc(reps: int = 1):
    if reps not in _NC_CACHE:
        _NC_CACHE[reps] = _build(reps=reps)
    return _NC_CACHE[reps]


class _CachedRunner:
    """Build the shard_map-jitted bass_exec callable once, reuse across calls."""

    def __init__(self, nc):
        import jax
        from jax.experimental.shard_map import shard_map
        from jax.sharding import Mesh, PartitionSpec
        from concourse import bass2jax, mybir as _mybir

        bass2jax.install_neuronx_cc_hook()
        self.nc = nc
        in_names, out_names, out_avals = [], [], []
        partition_name = (nc.partition_id_tensor.name
                          if nc.partition_id_tensor else None)
        for alloc in nc.m.functions[0].allocations:
            if not isinstance(alloc, _mybir.MemoryLocationSet):
                continue
            name = alloc.memorylocations[0].name
            if alloc.kind == "ExternalInput":
                if name != partition_name:
                    in_names.append(name)
            elif alloc.kind == "ExternalOutput":
                out_avals.append(jax.core.ShapedArray(
                    tuple(alloc.tensor_shape), _mybir.dt.np(alloc.dtype)))
                out_names.append(name)
        self.in_names, self.out_names, self.out_avals = in_names, out_names, out_avals
        n_params, n_outs = len(in_names), len(out_names)
        self.n_params = n_params
        all_names = in_names + out_names + ([partition_name] if partition_name else [])

        def _body(*args):
            operands = list(args)
            if partition_name is not None:
                operands.append(bass2jax.partition_id_tensor())
            return tuple(bass2jax._bass_exec_p.bind(
                *operands,
                out_avals=tuple(out_avals),
                in_names=tuple(all_names),
                out_names=tuple(out_names),
                lowering_input_output_aliases=(),
                sim_require_finite=True,
                sim_require_nnan=True,
                nc=nc,
            ))

        devices = jax.devices()[:N_CORES]
        mesh = Mesh(np.asarray(devices), ("core",))
        in_specs = (PartitionSpec("core"),) * (n_params + n_outs)
        out_specs = (PartitionSpec("core"),) * n_outs
        self.fn = jax.jit(
            shard_map(_body, mesh=mesh, in_specs=in_specs, out_specs=out_specs,
                      check_rep=False),
            donate_argnums=tuple(range(n_params, n_params + n_outs)),
            keep_unused=True,
        )

    def __call__(self, in_maps):
        concat_in = [
            np.concatenate([np.asarray(m[name]) for m in in_maps], axis=0)
            for name in self.in_names
        ]
        concat_zeros = [
            np.zeros((N_CORES * a.shape[0], *a.shape[1:]), a.dtype)
            for a in self.out_avals
        ]
        out_arrs = self.fn(*concat_in, *concat_zeros)
        import jax
        jax.block_until_ready(out_arrs)
        return [
            {name: np.asarray(out_arrs[i]).reshape(
                N_CORES, *self.out_avals[i].shape)[c]
             for i, name in enumerate(self.out_names)}
            for c in range(N_CORES)
        ]


_RUNNER_CACHE = {}


def _get_runner(reps: int = 1):
    if reps not in _RUNNER_CACHE:
        _RUNNER_CACHE[reps] = _CachedRunner(_get_nc(reps))
    return _RUNNER_CACHE[reps]


def _in_maps(p: np.ndarray, t: np.ndarray):
    t_bf = t.astype(ml_dtypes.bfloat16)
    p_bf = p.astype(ml_dtypes.bfloat16)
    p05_bf = (0.05 * p).astype(ml_dtypes.bfloat16)
    p05n_bf = (-0.05 * p).astype(ml_dtypes.bfloat16)
    ident = np.eye(128, dtype=ml_dtypes.bfloat16)
    in_maps = []
    for c in range(N_CORES):
        rs = slice(c * ROWS_PER_CORE, (c + 1) * ROWS_PER_CORE)
        in_maps.append({
            "tcol_bf": t_bf,
            "p05_bf": p05_bf, "p05n_bf": p05n_bf, "ident": ident,
            "prow": np.ascontiguousarray(p[rs]),
            "trow": np.ascontiguousarray(t[rs]),
        })
    return in_maps


def _run(nc, p: np.ndarray, t: np.ndarray):
    return run_bass_kernel_spmd(nc, _in_maps(p, t), core_ids=list(range(N_CORES)))


def _combine(results) -> np.float32:
    SA = 0.0
    SB = 0.0
    s_d2 = s_p2 = s_t2 = s_p = s_t = 0.0
    for r in results:
        acc = r["accC"].astype(np.float64)
        SA += acc[:, 0].sum()
        SB += acc[:, 1].sum()
        st = r["stats"].astype(np.float64)
        s_d2 += st[:, 0].sum()
        s_p2 += st[:, 1].sum()
        s_t2 += st[:, 2].sum()
        s_p += st[:, 3].sum()
        s_t += st[:, 4].sum()

    pair_count = N * (N - 1) // 2
    rank = (20.0 * SA + SB) / pair_count
    mse = s_d2 / N
    var_p = (s_p2 - s_p * s_p / N) / (N - 1)
    var_t = (s_t2 - s_t * s_t / N) / (N - 1)
    div = max(var_t - var_p, 0.0)
    return np.float32(MSE_W * mse + RANK_W * rank + DIV_W * div)


def kernel(predictions, targets) -> np.ndarray:
    p = np.asarray(predictions, dtype=np.float32)
    t = np.asarray(targets, dtype=np.float32)
    runner = _get_runner(reps=1)
    results = runner(_in_maps(p, t))
    out = _combine(results)
    return np.asarray(out, dtype=np.float32)
